# revision 13
# baseline (speedup 1.0000x reference)
"""Trainium2 Bass kernel for nn_AE_30142080483951 (gnn_message_passing).

Data-parallel over batch B=8 across 8 NeuronCores (one batch element per
core, weights replicated, no collectives).  Key restructuring vs the
reference:

  - The (M,M) affinity matrix A = SPf^T @ sigma @ SPf is rank-64, so
    A @ yT is computed as SPf^T @ (sigma @ (SPf @ yT)) without ever
    materializing A; the gnn linear is folded into the same low-rank chain.
  - softmax(sp_adj) is data-independent, precomputed on host, scaled by
    1024 into fp8e4m3, streamed in a chunk-major row-pair-interleaved
    layout for DoubleRow matmuls (1/1024 folded into sp_w).
  - SP is sent PIXEL-MAJOR: spt[p, ti, c, w] so the class-softmax sum is
    a free-axis vector reduce (no PE colsum matmuls, one tiny
    reciprocal), and the 2x2 maxpool's reduce over the window axis lands
    DIRECTLY in token-major layout (no SBUF-SBUF shuffle; feeds the
    g accumulation immediately).  normalize/pool chunks 1-2 run on
    gpsimd in parallel with vector's chunk 0.
  - g = SPf @ yT runs as 9 fp8 DoubleRow accumulations; yc/sigma/se stay
    bf16 (fp8 there costs ~4% relative error: random-sign dot products
    keep the full operand quantization error).
  - sigma/hg complete mid-kernel so the se tail is not end-gated; the
    3t residual is folded into the back conv as a second accumulation
    pass (kills one elementwise tail op per chunk).
  - bn1 scale folded into trans weights; engine programs ordered by data
    arrival (exp before trans evacuations on scalar).
  - A PE warmup spin during the input-DMA window ramps the tensor
    engine's DVFS p-state before the real matmuls arrive.
  - BatchNorms folded to per-channel scale/bias applied by ScalarE
    activations straight out of PSUM.  bf16 compute elsewhere (rel tol
    2e-2), fp32 PSUM accumulation, bf16 output store (f32 cast on host).
"""

import numpy as np
from contextlib import ExitStack

EPS = 1e-5
B, N, Cs, Cin, Ci, Co = 8, 48, 64, 256, 128, 128
M = N * N            # 2304
MT = M // 128        # 18 token tiles
HT2 = MT // 2        # 9 row-pair slices of the adjacency
CH = [(0, 512), (512, 512), (1024, 512), (1536, 512), (2048, 256)]
CHT = [(0, 4), (4, 4), (8, 4), (12, 4), (16, 2)]   # token tiles per chunk
ASP_SCALE = 1024.0   # host-side scale on softmax(sp_adj); folded into sp_w
N_WARM = 20          # PE p-state warmup matmuls

_CACHE = {}


def _build():
    import concourse.bacc as bacc_mod
    import concourse.mybir as mybir
    import concourse.tile as tile
    from concourse.bass import MemorySpace

    f32 = mybir.dt.float32
    bf = mybir.dt.bfloat16
    f8 = mybir.dt.float8e4
    AF = mybir.ActivationFunctionType
    DR = mybir.MatmulPerfMode.DoubleRow
    AX = mybir.AxisListType
    OP = mybir.AluOpType

    nc = bacc_mod.Bacc("TRN2", num_swdge_queues=4)

    # ---- DRAM parameters (per-core shard; bf16/fp8 matmul operands) ----
    x_d = nc.dram_tensor("x", [Cin, M], bf, kind="ExternalInput")
    # pixel-major SP: spt[p, ti*256 + c*4 + w] = SP[c, 2*rp+dy, 2*sp+dx],
    # token m = ti*128+p = rp*48+sp, w = dy*2+dx
    spt_d = nc.dram_tensor("spt", [128, MT * Cs * 4], bf, kind="ExternalInput")
    # host-softmaxed, x1024, fp8 adjacency, row-pair interleaved:
    # st8[i*128 + p, 2*m + k] = softmax(adj).T[256*i + 128*k + p, m] * 1024
    st8_d = nc.dram_tensor("st8", [M // 2, 2 * M], f8, kind="ExternalInput")
    w1t_d = nc.dram_tensor("w1t", [Cin, Ci], bf, kind="ExternalInput")
    wnct_d = nc.dram_tensor("wnct", [M, Cs], bf, kind="ExternalInput")
    bnc_d = nc.dram_tensor("bnc", [1, Cs], bf, kind="ExternalInput")
    # packed (Ci, 704) = [wkct(64) | gnnwt(128) | spwt(128) | backwt(128) | backwt3(128) | idb(128)]
    wpack_d = nc.dram_tensor("wpack", [Ci, 704], bf, kind="ExternalInput")
    # packed (Ci, 5) = [bn1b gnnb spb bn2s bn2b]
    bias_d = nc.dram_tensor("biases", [Ci, 5], f32, kind="ExternalInput")
    bkc_d = nc.dram_tensor("bkc", [Cs, 1], f32, kind="ExternalInput")
    out_d = nc.dram_tensor("out", [Co, M], bf, kind="ExternalOutput")

    tc = tile.TileContext(nc)
    with tc:
        with ExitStack() as ctx:
            ctx.enter_context(
                nc.allow_low_precision(reason="bf16/fp8 compute path, rel tol 2e-2")
            )
            singles = ctx.enter_context(tc.tile_pool(name="singles", bufs=1))
            tails = ctx.enter_context(tc.tile_pool(name="tails", bufs=3))
            psA = ctx.enter_context(
                tc.tile_pool(name="psA", bufs=1, space=MemorySpace.PSUM)
            )
            psS = ctx.enter_context(
                tc.tile_pool(name="psS", bufs=2, space=MemorySpace.PSUM)
            )

            with tc.tile_pool(name="phase1", bufs=1) as p1:
                # ---- DMA issue order is latency-priority ----
                # scalar queue: w1t (tiny, gates first LDWEIGHTS) then spT
                w1t_sb = p1.tile([128, 2, Ci], bf)
                nc.scalar.dma_start(
                    out=w1t_sb[:],
                    in_=w1t_d[:, :].rearrange("(kt p) c -> p kt c", p=128),
                )
                sp_sb = p1.tile([128, MT, Cs, 4], bf)
                nc.scalar.dma_start(
                    out=sp_sb[:],
                    in_=spt_d[:, :].rearrange("p (t c w) -> p t c w", t=MT, c=Cs),
                )
                # sync queue: x, then the fp8 adjacency in 3 slice-groups
                x_sb = p1.tile([128, 2, M], bf)
                nc.sync.dma_start(
                    out=x_sb[:], in_=x_d[:, :].rearrange("(kt p) m -> p kt m", p=128)
                )
                est_sb = singles.tile([128, HT2, 2 * M], f8)
                for gi in range(3):
                    nc.sync.dma_start(
                        out=est_sb[:, 3 * gi : 3 * (gi + 1), :],
                        in_=st8_d[384 * gi : 384 * (gi + 1), :].rearrange(
                            "(s p) m -> p s m", p=128
                        ),
                    )
                # gpsimd queue: small weights
                wpack_sb = singles.tile([Ci, 704], bf)
                nc.gpsimd.dma_start(out=wpack_sb[:], in_=wpack_d[:, :])
                wkct_sb = wpack_sb[:, 0:64]
                gnnwt_sb = wpack_sb[:, 64:192]
                spwt_sb = wpack_sb[:, 192:320]
                backwt_sb = wpack_sb[:, 320:448]
                backwt3_sb = wpack_sb[:, 448:576]
                idb = wpack_sb[:, 576:704]
                bias_sb = singles.tile([Ci, 5], f32)
                nc.gpsimd.dma_start(out=bias_sb[:], in_=bias_d[:, :])
                bn1b_sb = bias_sb[:, 0:1]
                gnnb_sb = bias_sb[:, 1:2]
                spb_sb = bias_sb[:, 2:3]
                bn2s_sb = bias_sb[:, 3:4]
                bn2b_sb = bias_sb[:, 4:5]
                bkc_sb = singles.tile([Cs, 1], f32)
                nc.gpsimd.dma_start(out=bkc_sb[:], in_=bkc_d[:, :])
                bnc_sb = singles.tile([1, Cs], bf)
                nc.gpsimd.dma_start(out=bnc_sb[:], in_=bnc_d[:, :])
                wnct_sb = p1.tile([128, MT, Cs], bf)
                nc.gpsimd.dma_start(
                    out=wnct_sb[:],
                    in_=wnct_d[:, :].rearrange("(ti p) c -> p ti c", p=128),
                )

                # persistent activations
                t_sb = singles.tile([Ci, M], bf)
                yT_sb = singles.tile([128, MT, Ci], bf)
                yT8_sb = singles.tile([128, MT, Ci], f8)
                spfT_sb = singles.tile([128, MT, Cs], bf)
                spfT8_sb = singles.tile([128, MT, Cs], f8)
                spfc_sb = singles.tile([Cs, M], bf)
                spre_sb = singles.tile([Ci, M], bf)
                hg_sb = singles.tile([Cs, Ci], bf)

                onesP = p1.tile([128, 128], bf)
                nc.vector.memset(onesP[:], 1.0)

                # ---- PE warmup spin: ramp DVFS while inputs stream in ----
                for wi in range(N_WARM):
                    wps = psS.tile([128, 128], f32, tag="ps_small")
                    nc.tensor.matmul(wps[:], onesP[:], onesP[:])

                # 5 PSUM banks shared by trans -> fp8 chain -> tails
                psb = [
                    psA.tile([128, 512], f32, tag=f"ps_chain{j}", name=f"psb{j}")
                    for j in range(len(CH))
                ]

                # ---- t = relu(bn1(W1 @ x)) (bn1 scale folded into w1t) ----
                for j, (mo, mw) in enumerate(CH):
                    nc.tensor.matmul(
                        psb[j][:, :mw],
                        w1t_sb[:, 0, :],
                        x_sb[:, 0, mo : mo + mw],
                        start=True,
                        stop=False,
                    )
                    nc.tensor.matmul(
                        psb[j][:, :mw],
                        w1t_sb[:, 1, :],
                        x_sb[:, 1, mo : mo + mw],
                        start=False,
                        stop=True,
                    )

                # ---- scalar program: exp chunks FIRST (sp path is the
                # critical chain), then per-chunk trans relu evacuations ----
                spv = sp_sb[:].rearrange("p t c w -> p (t c w)")
                for q in range(3):
                    qs = slice(q * 1536, (q + 1) * 1536)
                    nc.scalar.activation(spv[:, qs], spv[:, qs], AF.Exp)

                # ---- sp softmax path, pixel-major, chunked by 6 tiles ----
                # per-pixel class sums = free-axis reduce, tiny reciprocal,
                # normalize with stride-0 broadcast, pool over window axis
                # writes token-major spfT directly.
                # vector: sums+recips (all), normalize/pool chunk 0
                # gpsimd: normalize/pool chunks 1-2
                d_sb = p1.tile([128, MT, 1, 4], f32)
                dinv_sb = p1.tile([128, MT, 1, 4], f32)
                for q in range(3):
                    ts_ = slice(q * 6, (q + 1) * 6)
                    nc.vector.tensor_reduce(
                        out=d_sb[:, ts_, 0, :],
                        in_=sp_sb[:, ts_, :, :].rearrange("p t c w -> p t w c"),
                        axis=AX.X,
                        op=OP.add,
                    )
                    nc.vector.reciprocal_approx_fast(
                        dinv_sb[:, ts_, :, :].rearrange("p t one w -> p (t one w)"),
                        d_sb[:, ts_, :, :].rearrange("p t one w -> p (t one w)"),
                    )
                    nc.vector.tensor_tensor(
                        out=sp_sb[:, ts_, :, :],
                        in0=sp_sb[:, ts_, :, :],
                        in1=dinv_sb[:, ts_, :, :].broadcast_to([128, 6, Cs, 4]),
                        op=OP.mult,
                    )
                    nc.vector.tensor_reduce(
                        out=spfT_sb[:, ts_, :],
                        in_=sp_sb[:, ts_, :, :],
                        axis=AX.X,
                        op=OP.max,
                    )
                # fp8 shadow for the g DoubleRow accumulation
                nc.vector.tensor_copy(
                    spfT8_sb[:].rearrange("p t c -> p (t c)"),
                    spfT_sb[:].rearrange("p t c -> p (t c)"),
                )

                # ---- token transposes of t (bf16) with dual evacuation
                # (vector: bf16 yT for yc; scalar: fp8 yT8 for chain/g),
                # yc accumulation matmuls interleaved per tile ----
                ps_yc = psS.tile([128, Cs], f32, tag="ps_acc", bufs=1)
                for j, (mo, mw) in enumerate(CH):
                    nc.scalar.activation(
                        t_sb[:, mo : mo + mw],
                        psb[j][:, :mw],
                        AF.Relu,
                        bias=bn1b_sb,
                    )
                    for ti in range(CHT[j][0], CHT[j][0] + CHT[j][1]):
                        ps = psS.tile([128, 128], bf, tag="ps_small")
                        nc.tensor.transpose(
                            ps[:], t_sb[:, ti * 128 : (ti + 1) * 128], idb
                        )
                        nc.vector.tensor_copy(yT_sb[:, ti, :], ps[:])
                        nc.scalar.activation(yT8_sb[:, ti, :], ps[:], AF.Copy)
                        # yc accumulation lags one tile so the PE doesn't
                        # stall on the vector evacuation
                        if ti > 0:
                            nc.tensor.matmul(
                                ps_yc[:],
                                yT_sb[:, ti - 1, :],
                                wnct_sb[:, ti - 1, :],
                                start=(ti == 1),
                                stop=False,
                            )
                nc.tensor.matmul(
                    ps_yc[:],
                    yT_sb[:, MT - 1, :],
                    wnct_sb[:, MT - 1, :],
                    start=False,
                    stop=False,
                )
                nc.tensor.matmul(
                    ps_yc[:], onesP[0:1, :], bnc_sb[:], start=False, stop=True
                )
                yc_sb = p1.tile([Ci, Cs], bf)
                nc.vector.tensor_copy(yc_sb[:], ps_yc[:])

                ps_sg = psS.tile([Cs, Cs], f32, tag="ps_small")
                nc.tensor.matmul(ps_sg[:], wkct_sb, yc_sb[:])
                sigT_sb = p1.tile([Cs, Cs], bf)
                nc.scalar.activation(sigT_sb[:], ps_sg[:], AF.Identity, bias=bkc_sb[:])

                # ---- g (cs, ci) via 9 fp8 DoubleRow accumulations ----
                ps_g = psS.tile([Cs, Ci], f32, tag="ps_acc", bufs=1)
                for i in range(HT2):
                    nc.tensor.matmul(
                        ps_g[:],
                        spfT8_sb[:, 2 * i : 2 * i + 2, :],
                        yT8_sb[:, 2 * i : 2 * i + 2, :],
                        start=(i == 0),
                        stop=(i == HT2 - 1),
                        perf_mode=DR,
                    )
                g_sb = p1.tile([Cs, Ci], bf)
                nc.vector.tensor_copy(g_sb[:], ps_g[:])

                ps_ht = psS.tile([Ci, Cs], f32, tag="ps_small")
                nc.tensor.matmul(ps_ht[:], g_sb[:], sigT_sb[:])
                ht_sb = p1.tile([Ci, Cs], bf)
                nc.vector.tensor_copy(ht_sb[:], ps_ht[:])

                ps_hg = psS.tile([Cs, Ci], f32, tag="ps_small")
                nc.tensor.matmul(ps_hg[:], ht_sb[:], gnnwt_sb)
                nc.vector.tensor_copy(hg_sb[:], ps_hg[:])

                # ---- fp8 DoubleRow chain, i-major: one LDWEIGHTS per
                # row-pair slice serves all 5 PSUM banks.  The 18 spf
                # transposes (bf16; spfc channel-major for the se branch)
                # are interleaved into the later slices to fill est-DMA
                # wait gaps; their evacuations run on scalar. ----
                def chain_slice(i):
                    for j, (mo, mw) in enumerate(CH):
                        nc.tensor.matmul(
                            psb[j][:, :mw],
                            yT8_sb[:, 2 * i : 2 * i + 2, :],
                            est_sb[:, i, 2 * mo : 2 * (mo + mw)].rearrange(
                                "p (m two) -> p two m", two=2
                            ),
                            start=(i == 0),
                            stop=(i == HT2 - 1),
                            perf_mode=DR,
                        )

                def spf_transpose(ti):
                    ps = psS.tile([Cs, 128], bf, tag="ps_small")
                    nc.tensor.transpose(ps[:], spfT_sb[:, ti, :], idb)
                    nc.scalar.activation(
                        spfc_sb[:, ti * 128 : (ti + 1) * 128], ps[:], AF.Copy
                    )

                for i in range(3):
                    chain_slice(i)
                tpi = 0
                for i in range(3, HT2):
                    for _ in range(3):
                        spf_transpose(tpi)
                        tpi += 1
                    chain_slice(i)

                # ---- tails: sp linear, se branch, back conv with the 3t
                # residual folded in as a second accumulation pass ----
                def tail_a(j):
                    mo, mw = CH[j]
                    sl_ = slice(mo, mo + mw)
                    # spre holds 1024*(Asp @ yT); 1/1024 folded into spwt
                    nc.vector.tensor_copy(spre_sb[:, sl_], psb[j][:, :mw])
                    nc.tensor.matmul(psb[j][:, :mw], spwt_sb, spre_sb[:, sl_])
                    y3a = tails.tile([128, 512], bf, tag="y3a", bufs=5)
                    nc.scalar.activation(
                        y3a[:, :mw], psb[j][:, :mw], AF.Relu, bias=spb_sb
                    )
                    return y3a

                def tail_b(j, y3a):
                    mo, mw = CH[j]
                    sl_ = slice(mo, mo + mw)
                    nc.tensor.matmul(psb[j][:, :mw], hg_sb[:], spfc_sb[:, sl_])
                    rse = tails.tile([128, 512], bf, tag="rse")
                    nc.vector.tensor_scalar(
                        out=rse[:, :mw],
                        in0=psb[j][:, :mw],
                        scalar1=gnnb_sb,
                        scalar2=0.0,
                        op0=OP.add,
                        op1=OP.max,
                    )
                    y3b = tails.tile([128, 512], bf, tag="y3b")
                    nc.vector.tensor_add(y3b[:, :mw], y3a[:, :mw], rse[:, :mw])
                    nc.tensor.matmul(
                        psb[j][:, :mw], backwt_sb, y3b[:, :mw], start=True, stop=False
                    )
                    nc.tensor.matmul(
                        psb[j][:, :mw], backwt3_sb, t_sb[:, sl_], start=False, stop=True
                    )
                    ob = tails.tile([128, 512], bf, tag="ob")
                    nc.scalar.activation(
                        ob[:, :mw],
                        psb[j][:, :mw],
                        AF.Relu,
                        bias=bn2b_sb,
                        scale=bn2s_sb,
                    )
                    nc.gpsimd.dma_start(out=out_d[:, sl_], in_=ob[:, :mw])

                y3as = [tail_a(j) for j in range(len(CH))]
                for j in range(len(CH)):
                    tail_b(j, y3as[j])

    nc.finalize()
    return nc


def _host_prep(inputs):
    """Fold BNs, transpose weights, precompute softmax(sp_adj) (parameter-
    only), cast matmul operands to bf16/fp8, build the 8 per-core input
    maps (core b gets batch element b)."""
    import ml_dtypes

    f = np.float32
    bf = ml_dtypes.bfloat16
    f8 = ml_dtypes.float8_e4m3
    x = np.ascontiguousarray(inputs["x"], dtype=f).reshape(B, Cin, M)
    SP = np.ascontiguousarray(inputs["SP"], dtype=f)  # (B, Cs, 96, 96)

    bn1s = (np.asarray(inputs["bn1_gamma"]) / np.sqrt(np.asarray(inputs["bn1_var"]) + EPS)).astype(f)
    bn1b = (np.asarray(inputs["bn1_beta"]) - np.asarray(inputs["bn1_mean"]) * bn1s).astype(f)
    bn2s = (np.asarray(inputs["bn2_gamma"]) / np.sqrt(np.asarray(inputs["bn2_var"]) + EPS)).astype(f)
    bn2b = (np.asarray(inputs["bn2_beta"]) - np.asarray(inputs["bn2_mean"]) * bn2s).astype(f)

    # softmax over the last axis of the learned adjacency; row-pair
    # interleaved fp8 layout: st8[i*128+p, 2*m+k] = AspT[256i+128k+p, m]
    adj = np.asarray(inputs["sp_adj"], dtype=np.float64)
    e = np.exp(adj - adj.max(axis=1, keepdims=True))
    asp = e / e.sum(axis=1, keepdims=True)
    aspT = (asp.T * ASP_SCALE).astype(f)                      # (M, M)
    il = aspT.reshape(HT2, 2, 128, M).transpose(0, 2, 3, 1)   # (i, p, m, k)
    st8 = np.ascontiguousarray(il.reshape(M // 2, 2 * M)).astype(f8)

    wpack = np.concatenate(
        [
            np.asarray(inputs["linKC_w"]).T,                    # (128, 64)
            np.asarray(inputs["gnn_w"]).T,                      # (128, 128)
            np.asarray(inputs["sp_w"]).T / ASP_SCALE,           # (128, 128)
            np.asarray(inputs["back_w"]).T,                     # (128, 128)
            np.asarray(inputs["back_w"]).T * 3.0,               # (128, 128)
            np.eye(128, dtype=f),                               # (128, 128)
        ],
        axis=1,
    ).astype(bf)
    biases = np.stack([bn1b,
                       np.asarray(inputs["gnn_b"], dtype=f),
                       np.asarray(inputs["sp_b"], dtype=f),
                       bn2s, bn2b], axis=1).astype(f)

    # bn1 scale folded into trans weight
    w1t = (np.asarray(inputs["trans_w"]).T * bn1s[None, :]).astype(f)

    # pixel-major SP layout: [p, ti, c, w]
    # (B, 64, 96, 96) -> (B, rp, sp, c, dy, dx) -> (B, m, c, 4) -> (B, 128, 18*64*4)
    spt = SP.reshape(B, Cs, 48, 2, 48, 2).transpose(0, 2, 4, 1, 3, 5)
    spt = spt.reshape(B, M, Cs, 4).reshape(B, MT, 128, Cs * 4)
    spt = np.ascontiguousarray(spt.transpose(0, 2, 1, 3)).reshape(B, 128, MT * Cs * 4)

    shared = {
        "st8": st8,
        "w1t": np.ascontiguousarray(w1t).astype(bf),
        "wnct": np.ascontiguousarray(np.asarray(inputs["linNC_w"]).T).astype(bf),
        "bnc": np.asarray(inputs["linNC_b"], dtype=f).reshape(1, Cs).astype(bf),
        "wpack": np.ascontiguousarray(wpack),
        "biases": np.ascontiguousarray(biases),
        "bkc": np.asarray(inputs["linKC_b"], dtype=f).reshape(Cs, 1),
    }
    in_maps = []
    for b in range(B):
        m = dict(shared)
        m["x"] = np.ascontiguousarray(x[b]).astype(bf)
        m["spt"] = np.ascontiguousarray(spt[b]).astype(bf)
        in_maps.append(m)
    return in_maps


def _get_nc():
    if "nc" not in _CACHE:
        _CACHE["nc"] = _build()
    return _CACHE["nc"]


def run_spmd(inputs, trace=False, trace_cores=None):
    """Build (cached), run on cores 0-7, return BassKernelResults."""
    from concourse.bass_utils import run_bass_kernel_spmd

    nc = _get_nc()
    in_maps = _host_prep(inputs)
    kwargs = {}
    if trace:
        kwargs = dict(trace=True, trace_cores=trace_cores or [0])
    return run_bass_kernel_spmd(nc, in_maps, core_ids=list(range(8)), **kwargs)


def kernel(**inputs):
    res = run_spmd(inputs)
    out = np.stack([r["out"].reshape(Co, N, N) for r in res.results])
    return out.astype(np.float32)


# revision 15
# speedup vs baseline: 1.3385x; 1.3385x over previous
"""Trainium2 Bass kernel for nn_AE_30142080483951 (gnn_message_passing).

Data-parallel over batch B=8 across 8 NeuronCores (one batch element per
core, weights replicated, no collectives).  Key restructuring vs the
reference:

  - The (M,M) affinity matrix A = SPf^T @ sigma @ SPf is rank-64, so
    A @ yT is computed as SPf^T @ (sigma @ (SPf @ yT)) without ever
    materializing A; the gnn linear is folded into the same low-rank chain.
  - softmax(sp_adj) is data-independent, precomputed on host, scaled by
    1024 into fp8e4m3 (1/1024 folded into sp_w), streamed slice-major
    with the two DoubleRow K-rows in separate column halves.
  - SP is sent PIXEL-MAJOR [p, tile, window, class] so the class-softmax
    sum is a contiguous free-axis vector reduce (no PE colsum matmuls,
    one tiny reciprocal) and the 2x2 maxpool (two pairwise maxes) lands
    directly in token-major layout, feeding the g accumulation without
    any SBUF shuffle.
  - g = SPf @ yT runs as 9 fp8 DoubleRow accumulations; yc/sigma/se stay
    bf16 (fp8 there costs ~4% relative error: random-sign dot products
    keep the full operand quantization error).
  - sigma/hg complete mid-chain so the se tail is not end-gated; the
    3t residual is folded into the back conv as a second accumulation
    pass.
  - DMA queue plan: x alone on the scalar queue (fast kickoff, full
    early bandwidth -> PE starts ~12us), everything else on sync with
    per-slice adjacency triggers so the small early tensors are not
    starved by the 5.3MB stream; outputs ride the gpsimd queue.
  - Engine programs ordered by data arrival: exp chunks interleaved
    with trans relus on scalar, transpose evacuations + softmax chunks
    interleaved on vector; a PE warmup spin ramps the DVFS p-state
    during the input-DMA window.
  - BatchNorms folded to per-channel scale/bias applied by ScalarE
    activations straight out of PSUM (bn1 scale folded into the trans
    weights).  bf16 compute elsewhere (rel tol 2e-2), fp32 PSUM
    accumulation, bf16 output store (f32 cast on host).
"""

import numpy as np
from contextlib import ExitStack

EPS = 1e-5
B, N, Cs, Cin, Ci, Co = 8, 48, 64, 256, 128, 128
M = N * N            # 2304
MT = M // 128        # 18 token tiles
HT2 = MT // 2        # 9 row-pair slices of the adjacency
CH = [(0, 512), (512, 512), (1024, 512), (1536, 512), (2048, 256)]
CHT = [(0, 4), (4, 4), (8, 4), (12, 4), (16, 2)]   # token tiles per chunk
ASP_SCALE = 1024.0   # host-side scale on softmax(sp_adj); folded into sp_w
N_WARM = 14          # PE p-state warmup matmuls

_CACHE = {}


def _build():
    import concourse.bacc as bacc_mod
    import concourse.mybir as mybir
    import concourse.tile as tile
    from concourse.bass import MemorySpace

    f32 = mybir.dt.float32
    bf = mybir.dt.bfloat16
    f8 = mybir.dt.float8e4
    AF = mybir.ActivationFunctionType
    DR = mybir.MatmulPerfMode.DoubleRow
    AX = mybir.AxisListType
    OP = mybir.AluOpType

    nc = bacc_mod.Bacc("TRN2", num_swdge_queues=4)

    # ---- DRAM parameters (per-core shard; bf16/fp8 matmul operands) ----
    x_d = nc.dram_tensor("x", [Cin, M], bf, kind="ExternalInput")
    # pixel-major SP: spt[p, ti*256 + w*64 + c] = SP[c, 2*rp+dy, 2*sp+dx],
    # token m = ti*128+p = rp*48+sp, w = dy*2+dx
    spt_d = nc.dram_tensor("spt", [128, MT * Cs * 4], bf, kind="ExternalInput")
    # host-softmaxed, x1024, fp8 adjacency, split DoubleRow layout:
    # st8[i*128 + p, k*M + m] = softmax(adj).T[256*i + 128*k + p, m] * 1024
    st8_d = nc.dram_tensor("st8", [M // 2, 2 * M], f8, kind="ExternalInput")
    # packed (Ci, 960) = [wkct(64) | gnnwt(128) | spwt(128) | backwt(128)
    #                     | backwt3(128) | idb(128) | w1t_k0(128) | w1t_k1(128)]
    wpack_d = nc.dram_tensor("wpack", [Ci, 960], bf, kind="ExternalInput")
    # per-partition-contiguous linNC weight: wnct[p, ti*64+c] = linNC_w.T[ti*128+p, c]
    wnct_d = nc.dram_tensor("wnct", [128, MT * Cs], bf, kind="ExternalInput")
    # packed (Ci, 5) = [bn1b gnnb spb bn2s bn2b]
    bias_d = nc.dram_tensor("biases", [Ci, 5], f32, kind="ExternalInput")
    bkc_d = nc.dram_tensor("bkc", [Cs, 1], f32, kind="ExternalInput")
    bnc_d = nc.dram_tensor("bnc", [1, Cs], bf, kind="ExternalInput")
    out_d = nc.dram_tensor("out", [Co, M], bf, kind="ExternalOutput")

    tc = tile.TileContext(nc)
    with tc:
        with ExitStack() as ctx:
            ctx.enter_context(
                nc.allow_low_precision(reason="bf16/fp8 compute path, rel tol 2e-2")
            )
            singles = ctx.enter_context(tc.tile_pool(name="singles", bufs=1))
            tails = ctx.enter_context(tc.tile_pool(name="tails", bufs=3))
            psA = ctx.enter_context(
                tc.tile_pool(name="psA", bufs=1, space=MemorySpace.PSUM)
            )
            psS = ctx.enter_context(
                tc.tile_pool(name="psS", bufs=2, space=MemorySpace.PSUM)
            )

            with tc.tile_pool(name="phase1", bufs=1) as p1:
                # ---- DMA plan: x ALONE on the scalar queue (its ring kicks
                # off ~3us earlier and x takes the full early bandwidth);
                # sync carries weights + spT + the 9 adjacency slices ----
                x_sb = p1.tile([128, 2, M], bf)
                nc.scalar.dma_start(
                    out=x_sb[:], in_=x_d[:, :].rearrange("(kt p) m -> p kt m", p=128)
                )
                wpack_sb = singles.tile([Ci, 960], bf)
                nc.sync.dma_start(out=wpack_sb[:], in_=wpack_d[:, :])
                wkct_sb = wpack_sb[:, 0:64]
                gnnwt_sb = wpack_sb[:, 64:192]
                spwt_sb = wpack_sb[:, 192:320]
                backwt_sb = wpack_sb[:, 320:448]
                backwt3_sb = wpack_sb[:, 448:576]
                idb = wpack_sb[:, 576:704]
                w1t_sb = wpack_sb[:, 704:960].rearrange("p (kt c) -> p kt c", kt=2)
                sp_sb = p1.tile([128, MT, 4, Cs], bf)
                for q in range(3):
                    nc.sync.dma_start(
                        out=sp_sb[:, 6 * q : 6 * (q + 1), :, :],
                        in_=spt_d[:, 1536 * q : 1536 * (q + 1)].rearrange(
                            "p (t w c) -> p t w c", t=6, w=4
                        ),
                    )
                wnct_sb = p1.tile([128, MT, Cs], bf)
                nc.sync.dma_start(
                    out=wnct_sb[:],
                    in_=wnct_d[:, :].rearrange("p (t c) -> p t c", t=MT),
                )
                est_sb = singles.tile([128, HT2, 2, M], f8)
                for i in range(HT2):
                    nc.sync.dma_start(
                        out=est_sb[:, i, :, :],
                        in_=st8_d[128 * i : 128 * (i + 1), :].rearrange(
                            "p (two m) -> p two m", two=2
                        ),
                    )
                # gpsimd queue: small biases
                bias_sb = singles.tile([Ci, 5], f32)
                nc.gpsimd.dma_start(out=bias_sb[:], in_=bias_d[:, :])
                bn1b_sb = bias_sb[:, 0:1]
                gnnb_sb = bias_sb[:, 1:2]
                spb_sb = bias_sb[:, 2:3]
                bn2s_sb = bias_sb[:, 3:4]
                bn2b_sb = bias_sb[:, 4:5]
                bkc_sb = singles.tile([Cs, 1], f32)
                nc.gpsimd.dma_start(out=bkc_sb[:], in_=bkc_d[:, :])
                bnc_sb = singles.tile([1, Cs], bf)
                nc.gpsimd.dma_start(out=bnc_sb[:], in_=bnc_d[:, :])

                # persistent activations
                t_sb = singles.tile([Ci, M], bf)
                yT_sb = singles.tile([128, MT, Ci], bf)
                yT8_sb = singles.tile([128, MT, Ci], f8)
                spfT_sb = singles.tile([128, MT, Cs], bf)
                spfT8_sb = singles.tile([128, MT, Cs], f8)
                spfc_sb = singles.tile([Cs, M], bf)
                spre_sb = singles.tile([Ci, M], bf)
                hg_sb = singles.tile([Cs, Ci], bf)

                onesP = p1.tile([128, 128], bf)
                nc.vector.memset(onesP[:], 1.0)

                # ---- PE warmup spin: ramp DVFS while x streams in ----
                for wi in range(N_WARM):
                    wps = psS.tile([128, 128], f32, tag="ps_small")
                    nc.tensor.matmul(wps[:], onesP[:], onesP[:])

                # 5 PSUM banks shared by trans -> fp8 chain -> tails
                psb = [
                    psA.tile([128, 512], f32, tag=f"ps_chain{j}", name=f"psb{j}")
                    for j in range(len(CH))
                ]

                # ---- t = relu(bn1(W1 @ x)) (bn1 scale folded into w1t) ----
                for j, (mo, mw) in enumerate(CH):
                    nc.tensor.matmul(
                        psb[j][:, :mw],
                        w1t_sb[:, 0, :],
                        x_sb[:, 0, mo : mo + mw],
                        start=True,
                        stop=False,
                    )
                    nc.tensor.matmul(
                        psb[j][:, :mw],
                        w1t_sb[:, 1, :],
                        x_sb[:, 1, mo : mo + mw],
                        start=False,
                        stop=True,
                    )

                # ---- scalar program: exp chunks interleaved with the trans
                # relu evacuations (exp gates the sp path, relus gate the
                # token transposes; neither may monopolize the engine) ----
                spv = sp_sb[:].rearrange("p t w c -> p (t w c)")

                def exp_chunk(q):
                    qs = slice(q * 1536, (q + 1) * 1536)
                    nc.scalar.activation(spv[:, qs], spv[:, qs], AF.Exp)

                def relu_chunk(j):
                    mo, mw = CH[j]
                    nc.scalar.activation(
                        t_sb[:, mo : mo + mw], psb[j][:, :mw], AF.Relu, bias=bn1b_sb
                    )

                exp_chunk(0)
                relu_chunk(0)
                relu_chunk(1)
                exp_chunk(1)
                relu_chunk(2)
                relu_chunk(3)
                exp_chunk(2)
                relu_chunk(4)

                # ---- sp softmax path, pixel-major [p, t, w, c] (contiguous
                # class axis), chunked by 6 tiles: per-pixel class sums =
                # contiguous X-reduce, tiny reciprocal, normalize with a
                # stride-0-broadcast multiply, 2x2 pool = two pairwise
                # maxes landing token-major ----
                d_sb = p1.tile([128, MT, 4], f32)
                dinv_sb = p1.tile([128, MT, 4], f32)
                for q in range(3):
                    ts_ = slice(q * 6, (q + 1) * 6)
                    nc.vector.tensor_reduce(
                        out=d_sb[:, ts_, :],
                        in_=sp_sb[:, ts_, :, :],
                        axis=AX.X,
                        op=OP.add,
                    )
                    nc.vector.reciprocal_approx_fast(
                        dinv_sb[:, ts_, :].rearrange("p t w -> p (t w)"),
                        d_sb[:, ts_, :].rearrange("p t w -> p (t w)"),
                    )
                    nc.vector.tensor_tensor(
                        out=sp_sb[:, ts_, :, :],
                        in0=sp_sb[:, ts_, :, :],
                        in1=dinv_sb[:, ts_, :]
                        .rearrange("p t (w one) -> p t w one", one=1)
                        .broadcast_to([128, 6, 4, Cs]),
                        op=OP.mult,
                    )
                    nc.vector.tensor_tensor(
                        out=sp_sb[:, ts_, 0:2, :],
                        in0=sp_sb[:, ts_, 0:2, :],
                        in1=sp_sb[:, ts_, 2:4, :],
                        op=OP.max,
                    )
                    nc.vector.tensor_tensor(
                        out=spfT_sb[:, ts_, :],
                        in0=sp_sb[:, ts_, 0:1, :].rearrange("p t one c -> p t (one c)"),
                        in1=sp_sb[:, ts_, 1:2, :].rearrange("p t one c -> p t (one c)"),
                        op=OP.max,
                    )
                # fp8 shadow for the g DoubleRow accumulation
                nc.vector.tensor_copy(
                    spfT8_sb[:].rearrange("p t c -> p (t c)"),
                    spfT_sb[:].rearrange("p t c -> p (t c)"),
                )

                # ---- token transposes of t (bf16): vector evacuates bf16
                # yT tiles; fp8 yT8 produced by 3 big vector casts; yc
                # accumulation matmuls interleaved one tile behind ----
                ps_yc = psS.tile([128, Cs], f32, tag="ps_acc", bufs=1)
                for ti in range(MT):
                    ps = psS.tile([128, 128], bf, tag="ps_small")
                    nc.tensor.transpose(
                        ps[:], t_sb[:, ti * 128 : (ti + 1) * 128], idb
                    )
                    nc.vector.tensor_copy(yT_sb[:, ti, :], ps[:])
                    if ti % 6 == 5:
                        nc.vector.tensor_copy(
                            yT8_sb[:, ti - 5 : ti + 1, :].rearrange(
                                "p t c -> p (t c)"
                            ),
                            yT_sb[:, ti - 5 : ti + 1, :].rearrange("p t c -> p (t c)"),
                        )
                    if ti > 0:
                        nc.tensor.matmul(
                            ps_yc[:],
                            yT_sb[:, ti - 1, :],
                            wnct_sb[:, ti - 1, :],
                            start=(ti == 1),
                            stop=False,
                        )
                nc.tensor.matmul(
                    ps_yc[:],
                    yT_sb[:, MT - 1, :],
                    wnct_sb[:, MT - 1, :],
                    start=False,
                    stop=False,
                )
                nc.tensor.matmul(
                    ps_yc[:], onesP[0:1, :], bnc_sb[:], start=False, stop=True
                )
                yc_sb = p1.tile([Ci, Cs], bf)
                nc.vector.tensor_copy(yc_sb[:], ps_yc[:])

                ps_sg = psS.tile([Cs, Cs], f32, tag="ps_small")
                nc.tensor.matmul(ps_sg[:], wkct_sb, yc_sb[:])
                sigT_sb = p1.tile([Cs, Cs], bf)
                nc.scalar.activation(sigT_sb[:], ps_sg[:], AF.Identity, bias=bkc_sb[:])

                # ---- fp8 DoubleRow chain, i-major: one LDWEIGHTS per
                # row-pair slice serves all 5 PSUM banks ----
                def chain_slice(i):
                    for j, (mo, mw) in enumerate(CH):
                        nc.tensor.matmul(
                            psb[j][:, :mw],
                            yT8_sb[:, 2 * i : 2 * i + 2, :],
                            est_sb[:, i, :, mo : mo + mw],
                            start=(i == 0),
                            stop=(i == HT2 - 1),
                            perf_mode=DR,
                        )

                for i in range(3):
                    chain_slice(i)

                # ---- g (cs, ci) via 9 fp8 DoubleRow accumulations, then
                # ht/hg: the se tail inputs are ready mid-chain ----
                ps_g = psS.tile([Cs, Ci], f32, tag="ps_acc", bufs=1)
                for i in range(HT2):
                    nc.tensor.matmul(
                        ps_g[:],
                        spfT8_sb[:, 2 * i : 2 * i + 2, :],
                        yT8_sb[:, 2 * i : 2 * i + 2, :],
                        start=(i == 0),
                        stop=(i == HT2 - 1),
                        perf_mode=DR,
                    )
                g_sb = p1.tile([Cs, Ci], bf)
                nc.vector.tensor_copy(g_sb[:], ps_g[:])

                ps_ht = psS.tile([Ci, Cs], f32, tag="ps_small")
                nc.tensor.matmul(ps_ht[:], g_sb[:], sigT_sb[:])
                ht_sb = p1.tile([Ci, Cs], bf)
                nc.vector.tensor_copy(ht_sb[:], ps_ht[:])

                ps_hg = psS.tile([Cs, Ci], f32, tag="ps_small")
                nc.tensor.matmul(ps_hg[:], ht_sb[:], gnnwt_sb)
                nc.vector.tensor_copy(hg_sb[:], ps_hg[:])

                # remaining chain slices with the 18 spf transposes (bf16,
                # channel-major spfc for the se branch; scalar evacuates)
                # interleaved to fill adjacency-DMA wait gaps
                def spf_transpose(ti):
                    ps = psS.tile([Cs, 128], bf, tag="ps_small")
                    nc.tensor.transpose(ps[:], spfT_sb[:, ti, :], idb)
                    nc.scalar.activation(
                        spfc_sb[:, ti * 128 : (ti + 1) * 128], ps[:], AF.Copy
                    )

                tpi = 0
                for i in range(3, HT2):
                    for _ in range(3):
                        spf_transpose(tpi)
                        tpi += 1
                    chain_slice(i)

                # ---- tails: sp linear, se branch, back conv with the 3t
                # residual folded in as a second accumulation pass ----
                def tail_a(j):
                    mo, mw = CH[j]
                    sl_ = slice(mo, mo + mw)
                    # spre holds 1024*(Asp @ yT); 1/1024 folded into spwt
                    if j % 2 == 0:
                        nc.scalar.activation(spre_sb[:, sl_], psb[j][:, :mw], AF.Copy)
                    else:
                        nc.vector.tensor_copy(spre_sb[:, sl_], psb[j][:, :mw])
                    nc.tensor.matmul(psb[j][:, :mw], spwt_sb, spre_sb[:, sl_])
                    y3a = tails.tile([128, 512], bf, tag="y3a", bufs=5)
                    nc.scalar.activation(
                        y3a[:, :mw], psb[j][:, :mw], AF.Relu, bias=spb_sb
                    )
                    return y3a

                def tail_b(j, y3a):
                    mo, mw = CH[j]
                    sl_ = slice(mo, mo + mw)
                    nc.tensor.matmul(psb[j][:, :mw], hg_sb[:], spfc_sb[:, sl_])
                    rse = tails.tile([128, 512], bf, tag="rse")
                    nc.vector.tensor_scalar(
                        out=rse[:, :mw],
                        in0=psb[j][:, :mw],
                        scalar1=gnnb_sb,
                        scalar2=0.0,
                        op0=OP.add,
                        op1=OP.max,
                    )
                    y3b = tails.tile([128, 512], bf, tag="y3b")
                    nc.vector.tensor_add(y3b[:, :mw], y3a[:, :mw], rse[:, :mw])
                    nc.tensor.matmul(
                        psb[j][:, :mw], backwt_sb, y3b[:, :mw], start=True, stop=False
                    )
                    nc.tensor.matmul(
                        psb[j][:, :mw], backwt3_sb, t_sb[:, sl_], start=False, stop=True
                    )
                    ob = tails.tile([128, 512], bf, tag="ob")
                    nc.scalar.activation(
                        ob[:, :mw],
                        psb[j][:, :mw],
                        AF.Relu,
                        bias=bn2b_sb,
                        scale=bn2s_sb,
                    )
                    nc.gpsimd.dma_start(out=out_d[:, sl_], in_=ob[:, :mw])

                y3as = [tail_a(j) for j in range(len(CH))]
                for j in range(len(CH)):
                    tail_b(j, y3as[j])

    nc.finalize()
    return nc


def _host_prep(inputs):
    """Fold BNs, transpose weights, precompute softmax(sp_adj) (parameter-
    only), cast matmul operands to bf16/fp8, build the 8 per-core input
    maps (core b gets batch element b)."""
    import ml_dtypes

    f = np.float32
    bf = ml_dtypes.bfloat16
    f8 = ml_dtypes.float8_e4m3
    x = np.ascontiguousarray(inputs["x"], dtype=f).reshape(B, Cin, M)
    SP = np.ascontiguousarray(inputs["SP"], dtype=f)  # (B, Cs, 96, 96)

    bn1s = (np.asarray(inputs["bn1_gamma"]) / np.sqrt(np.asarray(inputs["bn1_var"]) + EPS)).astype(f)
    bn1b = (np.asarray(inputs["bn1_beta"]) - np.asarray(inputs["bn1_mean"]) * bn1s).astype(f)
    bn2s = (np.asarray(inputs["bn2_gamma"]) / np.sqrt(np.asarray(inputs["bn2_var"]) + EPS)).astype(f)
    bn2b = (np.asarray(inputs["bn2_beta"]) - np.asarray(inputs["bn2_mean"]) * bn2s).astype(f)

    # softmax over the last axis of the learned adjacency; split DoubleRow
    # fp8 layout: st8[i*128+p, k*M+m] = AspT[256i+128k+p, m]
    adj = np.asarray(inputs["sp_adj"], dtype=np.float64)
    e = np.exp(adj - adj.max(axis=1, keepdims=True))
    asp = e / e.sum(axis=1, keepdims=True)
    aspT = (asp.T * ASP_SCALE).astype(f)                      # (M, M)
    spl = aspT.reshape(HT2, 2, 128, M).transpose(0, 2, 1, 3)  # (i, p, k, m)
    st8 = np.ascontiguousarray(spl.reshape(M // 2, 2 * M)).astype(f8)

    # bn1 scale folded into trans weight
    w1t = (np.asarray(inputs["trans_w"]).T * bn1s[None, :]).astype(f)  # (Cin, Ci)

    wpack = np.concatenate(
        [
            np.asarray(inputs["linKC_w"]).T,                    # (128, 64)
            np.asarray(inputs["gnn_w"]).T,                      # (128, 128)
            np.asarray(inputs["sp_w"]).T / ASP_SCALE,           # (128, 128)
            np.asarray(inputs["back_w"]).T,                     # (128, 128)
            np.asarray(inputs["back_w"]).T * 3.0,               # (128, 128)
            np.eye(128, dtype=f),                               # (128, 128)
            w1t[0:128, :],                                      # w1t k-rows 0-127
            w1t[128:256, :],                                    # w1t k-rows 128-255
        ],
        axis=1,
    ).astype(bf)

    biases = np.stack([bn1b,
                       np.asarray(inputs["gnn_b"], dtype=f),
                       np.asarray(inputs["sp_b"], dtype=f),
                       bn2s, bn2b], axis=1).astype(f)

    # per-partition contiguous linNC weight: wnct[p, ti*64+c]
    wnct = np.asarray(inputs["linNC_w"]).T.reshape(MT, 128, Cs)
    wnct = np.ascontiguousarray(wnct.transpose(1, 0, 2)).reshape(128, MT * Cs)

    # pixel-major SP layout: [p, ti, w, c]
    spt = SP.reshape(B, Cs, 48, 2, 48, 2).transpose(0, 2, 4, 3, 5, 1)
    spt = spt.reshape(B, M, 4 * Cs).reshape(B, MT, 128, 4 * Cs)
    spt = np.ascontiguousarray(spt.transpose(0, 2, 1, 3)).reshape(B, 128, MT * Cs * 4)

    shared = {
        "st8": st8,
        "wpack": np.ascontiguousarray(wpack),
        "wnct": np.ascontiguousarray(wnct).astype(bf),
        "biases": np.ascontiguousarray(biases),
        "bkc": np.asarray(inputs["linKC_b"], dtype=f).reshape(Cs, 1),
        "bnc": np.asarray(inputs["linNC_b"], dtype=f).reshape(1, Cs).astype(bf),
    }
    in_maps = []
    for b in range(B):
        m = dict(shared)
        m["x"] = np.ascontiguousarray(x[b]).astype(bf)
        m["spt"] = np.ascontiguousarray(spt[b]).astype(bf)
        in_maps.append(m)
    return in_maps


def _get_nc():
    if "nc" not in _CACHE:
        _CACHE["nc"] = _build()
    return _CACHE["nc"]


def run_spmd(inputs, trace=False, trace_cores=None):
    """Build (cached), run on cores 0-7, return BassKernelResults."""
    from concourse.bass_utils import run_bass_kernel_spmd

    nc = _get_nc()
    in_maps = _host_prep(inputs)
    kwargs = {}
    if trace:
        kwargs = dict(trace=True, trace_cores=trace_cores or [0])
    return run_bass_kernel_spmd(nc, in_maps, core_ids=list(range(8)), **kwargs)


def kernel(**inputs):
    res = run_spmd(inputs)
    out = np.stack([r["out"].reshape(Co, N, N) for r in res.results])
    return out.astype(np.float32)
